# revision 54
# baseline (speedup 1.0000x reference)
"""Causal single-head attention on 8 TRN2 NeuronCores.

Data-parallel over batch: core b computes attention for batch element b.

Streaming structure: the kernel processes q-chunks of the sequence in
causal order (a 256/256/512/512/512 chunk table -- the small head
chunks start the ScalarE EXP stream several us earlier, and exp is
ScalarE-only at 1 elem/cycle/lane, the true roofline of this kernel).
For each chunk it projects q/k/v and immediately runs the flash loop of
that q-chunk against all key tiles <= the diagonal, while later chunks'
xT is still in flight. Next-chunk projections are emitted interleaved
between flash pairs, boosted with tc.high_priority so the Tile
scheduler keeps the qk -> A/B -> scores -> EXP chain ahead of flexible
work (V projection, transposes, PV, masks).

Device output is the UN-normalized attention in e-major layout
[E+1, SEQ] bf16: rows 0..63 are sum_k exp(s)*v, row 64 is the softmax
denominator (from a ones-column appended to V). The host divides and
transposes -- no output transposes, reciprocals or normalization on
the device critical path.

Key layout tricks:
- [Wq|Wk] packed into one 128-col stationary: projection produces qT on
  partitions 0-63 and kT on partitions 64-127 (tensor A); a
  partition-swapped mirror (tensor B) is made with DVE copies (B-hi
  first -- the only new-B piece the chunk's first score pair needs).
- Score matmuls contract over e=64: pairs of key tiles run concurrently
  in PE row-groups (0,0) and (64,0), fed from A/B at matching base
  partitions.
- exp runs once per pair on ScalarE; causal masking is applied in-place
  on the exp output by GpSimd affine_select. The second diagonal pair
  of 512-chunks only computes/exps/PV-accumulates its live 256 columns.
- PV matmuls are delayed by one pair so exp/mask latency never stalls
  the PE FIFO; the denominator comes free from the ones-column of V.
- xT arrives partition-major, one contiguous block per chunk (large DMA
  descriptors); weights are one packed [p, dt, Wq|Wk|Wv] tensor.
- Outputs go out on the GpSimd software DMA ring, off the input rings.
"""

import os
import sys

sys.path.insert(0, "/opt/trn_rl_repo")

import numpy as np

BS, SEQ, D, E = 8, 2048, 768, 64
P = 128                  # SBUF partitions
N_DT = D // P            # 6 contraction tiles for the projections
N_KT = SEQ // P          # 16 key tiles
SCALE = 1.0 / np.sqrt(E).astype(np.float32)  # 0.125

# q-chunk table (start, size): small head chunks for an early EXP start
CH = [(0, 256), (256, 256), (512, 512), (1024, 512), (1536, 512)]
N_CH = len(CH)

DT_MM_NAME = os.environ.get("ATTN_DT_MM", "bfloat16")

_CACHE = {}

LAST_RESULT = None  # BassKernelResults of the most recent run (for test.py)


def _build(dt_mm_name):
    from contextlib import ExitStack

    import concourse.bass as bass  # noqa: F401
    import concourse.tile as tile
    from concourse import bacc, mybir
    from concourse.masks import make_identity

    f32 = mybir.dt.float32
    dt_in = getattr(mybir.dt, dt_mm_name)

    nc = bacc.Bacc(
        "TRN2", target_bir_lowering=False, debug=False, num_devices=BS
    )
    # per-chunk xT blocks, partition-major contiguous: [p, dt, s-in-chunk]
    xc_d = [
        nc.dram_tensor(
            "xc%d" % i, [P, N_DT, sz], dt_in, kind="ExternalInput"
        ).ap()
        for i, (st_, sz) in enumerate(CH)
    ]
    # weights packed partition-major: [p, dt, Wq|Wk|Wv]
    w_d = nc.dram_tensor(
        "W", [P, N_DT, 2 * E + E], dt_in, kind="ExternalInput"
    ).ap()
    # un-normalized e-major output + denominator row
    out_d = nc.dram_tensor(
        "out", [E + 1, SEQ], dt_in, kind="ExternalOutput"
    ).ap()

    with tile.TileContext(nc) as tc, ExitStack() as ctx:
        const = ctx.enter_context(tc.tile_pool(name="const", bufs=1))
        mm_ps = ctx.enter_context(tc.tile_pool(name="mm_ps", bufs=2, space="PSUM"))
        pv_ps = ctx.enter_context(tc.tile_pool(name="pv_ps", bufs=2, space="PSUM"))
        qk_ps = ctx.enter_context(tc.tile_pool(name="qk_ps", bufs=2, space="PSUM"))
        p_pool = ctx.enter_context(tc.tile_pool(name="p_pool", bufs=8))
        sc_pool = ctx.enter_context(tc.tile_pool(name="sc_pool", bufs=2))
        o_pool = ctx.enter_context(tc.tile_pool(name="o_pool", bufs=4))

        ident_mm = const.tile([P, P], dt_in)
        make_identity(nc, ident_mm)

        # --- warmup: dummy EXP forces the ACT table load during the DMA
        # phase; dummy matmuls keep the PE HAM busy so real matmuls start
        # at full clock ---
        zeros_sb = const.tile([P, 512], dt_in, tag="zeros")
        nc.vector.memset(zeros_sb[:], 0.0)
        for _ in range(9):
            dummy_ps = qk_ps.tile([P, 512], f32, tag="pj")
            nc.tensor.matmul(
                dummy_ps,
                lhsT=zeros_sb[:, 0:P],
                rhs=zeros_sb[:],
                start=True,
                stop=True,
            )

        # --- input DMAs: head chunks first on the sync ring, packed
        # weights lead the scalar ring, chunk 2 biased to scalar (it is
        # lighter early on), later chunks split across both rings ---
        w_sb = const.tile([P, N_DT, 2 * E + E], dt_in, tag="w")
        xc_sb = [
            const.tile([P, N_DT, sz], dt_in, tag="x%d" % i, name="x%d" % i)
            for i, (st_, sz) in enumerate(CH)
        ]
        DH = N_DT // 2
        nc.sync.dma_start(xc_sb[0][:], xc_d[0])
        nc.sync.dma_start(xc_sb[1][:], xc_d[1])
        nc.scalar.dma_start(w_sb[:], w_d)
        warm_sb = const.tile([P, 8], dt_in, tag="warm")
        nc.scalar.activation(
            warm_sb, zeros_sb[:, 0:8], mybir.ActivationFunctionType.Exp
        )
        nc.sync.dma_start(xc_sb[2][:, 0:2], xc_d[2][:, 0:2])
        nc.scalar.dma_start(xc_sb[2][:, 2:], xc_d[2][:, 2:])
        for i in (3, 4):
            nc.sync.dma_start(xc_sb[i][:, 0:DH], xc_d[i][:, 0:DH])
            nc.scalar.dma_start(xc_sb[i][:, DH:], xc_d[i][:, DH:])

        # SBUF persistent tensors
        A_sb = const.tile([P, SEQ], dt_in, tag="A")    # qT | kT
        B_sb = const.tile([P, SEQ], dt_in, tag="B")    # kT | qT (swap of A)
        v_sb = const.tile([P, N_KT, E + 1], dt_in, tag="v")
        nc.vector.memset(v_sb[:], 1.0)  # col E stays 1.0 = denominator

        # --- PV bookkeeping: delay PV matmuls by one pair ---
        pending = None  # (pv, l0, r0, l1, r1, start, stop)

        def flush_pending():
            nonlocal pending
            if pending is None:
                return
            pv_, l0, r0_, l1, r1, st_, sp_ = pending
            nc.tensor.matmul(pv_, lhsT=l0, rhs=r0_, start=st_, stop=False)
            nc.tensor.matmul(pv_, lhsT=l1, rhs=r1, start=False, stop=sp_)
            pending = None

        out_queue = []  # (start, size, pv) fully-accumulated chunks

        def flush_out():
            while out_queue:
                st_, sz, pv_ = out_queue.pop(0)
                o_sb = o_pool.tile(
                    [E + 1, 512], dt_in, tag="o", name="o"
                )[:, 0:sz]
                nc.vector.tensor_copy(o_sb, pv_[:])
                nc.gpsimd.dma_start(out_d[:, st_:st_ + sz], o_sb)

        qk_st = {}
        v_st = {}

        def emit_proj_qk(ci, dlo, dhi):
            """qk projection matmuls for d-tiles [dlo, dhi) of chunk ci;
            when dhi == N_DT also emit the A/B copies (A on ScalarE
            while ACT has stalls at these boundaries, DVE for the last
            chunk). Boosted: this chain gates the EXP stream."""
            st_, sz = CH[ci]
            csl = slice(st_, st_ + sz)
            with tc.high_priority(1000000):
                if dlo == 0:
                    qk_st[ci] = qk_ps.tile(
                        [P, 512], f32, tag="pj", name="pjqk"
                    )
                ps = qk_st[ci][:, 0:sz]
                for d in range(dlo, dhi):
                    nc.tensor.matmul(
                        ps,
                        lhsT=w_sb[:, d, 0:2 * E],
                        rhs=xc_sb[ci][:, d, :],
                        start=(d == 0),
                        stop=(d == N_DT - 1),
                    )
                if dhi == N_DT:
                    if ci <= 3:
                        nc.scalar.copy(A_sb[:, csl], ps)
                    else:
                        nc.vector.tensor_copy(A_sb[:, csl], ps)
                    # B-hi (qT at base 64) first: it is the only piece
                    # of the new B that the chunk's first score pair
                    # needs (its kT tiles were mirrored chunks ago)
                    nc.vector.tensor_copy(B_sb[E:P, csl], A_sb[0:E, csl])
                    nc.vector.tensor_copy(B_sb[0:E, csl], A_sb[E:P, csl])

        def emit_proj_v_mm(ci):
            st_, sz = CH[ci]
            psv = qk_ps.tile([E, 512], f32, tag="pj", name="pjv")[:, 0:sz]
            v_st[ci] = psv
            for d in range(N_DT):
                nc.tensor.matmul(
                    psv,
                    lhsT=w_sb[:, d, 2 * E:2 * E + E],
                    rhs=xc_sb[ci][:, d, :],
                    start=(d == 0),
                    stop=(d == N_DT - 1),
                )

        def emit_proj_v_tr(ci):
            st_, sz = CH[ci]
            nk = sz // P
            vT_sc = sc_pool.tile([E, 512], dt_in, tag="vT", name="vT")[:, 0:sz]
            nc.vector.tensor_copy(vT_sc, v_st[ci])
            vt = qk_ps.tile([P, 4, E], dt_in, tag="pj", name="pjvt")[:, 0:nk]
            for t in range(nk):
                nc.tensor.transpose(
                    vt[:, t, :],
                    vT_sc[:, t * P:(t + 1) * P],
                    ident_mm[0:E, 0:E],
                )
            nc.vector.tensor_copy(
                v_sb[:, st_ // P:st_ // P + nk, 0:E], vt
            )

        emit_proj_qk(0, 0, N_DT)
        for ci, (cst, csz) in enumerate(CH):
            # --- flash loop for q-chunk ci: key tiles 0..(cst+csz)/P-1
            # in pairs. Last chunk: diagonal pairs first, so the kernel
            # tail (last EXP -> PV -> copy -> DMA) has no mask dep ---
            n_pairs = (cst + csz) // P // 2
            order = list(range(n_pairs))
            if ci == N_CH - 1:
                order = order[-2:] + order[:-2]
            pv = pv_ps.tile([E + 1, 512], f32, tag="pv", name="pv")[:, 0:csz]
            for oi, pi in enumerate(order):
                t0, t1 = 2 * pi, 2 * pi + 1
                doff = t0 * P - cst   # >= 0 for diagonal pairs
                # second diag pair of a 512-chunk: cols < 256 are dead
                trim = 256 if doff == 256 and csz == 512 else 0
                with tc.high_priority(1000000):
                    s2 = mm_ps.tile([P, 2, 512], f32, tag="mm", name="s2")
                    s2v = s2[:, :, trim:csz]
                    # row-group (0,0): kT/qT from base partition 0
                    nc.tensor.matmul(
                        s2v[:, 0, :],
                        lhsT=B_sb[0:E, t0 * P:(t0 + 1) * P],
                        rhs=A_sb[0:E, cst + trim:cst + csz],
                        start=True,
                        stop=True,
                    )
                    # row-group (64,0): kT/qT from base partition 64
                    nc.tensor.matmul(
                        s2v[:, 1, :],
                        lhsT=A_sb[E:P, t1 * P:(t1 + 1) * P],
                        rhs=B_sb[E:P, cst + trim:cst + csz],
                        start=True,
                        stop=True,
                    )
                flush_pending()
                flush_out()
                p2 = p_pool.tile([P, 2, 512], dt_in, tag="p", name="p2")
                with tc.high_priority(1000000):
                    nc.scalar.activation(
                        p2[:, :, trim:csz],
                        s2v[:, :, :],
                        mybir.ActivationFunctionType.Exp,
                        scale=float(SCALE),
                    )
                if doff >= 0:
                    # zero entries with k > q, in place on the exp
                    # output: keep elem[part, i, y] iff
                    #   (y + cst + trim) >= part + (t0 + i)*P
                    nc.gpsimd.affine_select(
                        out=p2[:, :, trim:csz],
                        in_=p2[:, :, trim:csz],
                        compare_op=mybir.AluOpType.is_ge,
                        fill=0.0,
                        base=trim - doff,
                        pattern=[[-P, 2], [1, csz - trim]],
                        channel_multiplier=-1,
                    )
                # trimmed pair: PV only accumulates its live columns,
                # so the un-exp'd region of p2 is never read
                pending = (
                    pv[:, trim:csz],
                    v_sb[:, t0, :],
                    p2[:, 0, trim:csz],
                    v_sb[:, t1, :],
                    p2[:, 1, trim:csz],
                    oi == 0,
                    oi == n_pairs - 1,
                )
                # next-chunk projections interleaved between pairs so
                # they fill EXP-paced PE idle without displacing score
                # matmuls (head chunks pull their own V projection
                # behind the first score pair)
                if ci == 0:
                    if oi == 0:
                        emit_proj_v_mm(0)
                        emit_proj_v_tr(0)
                        emit_proj_qk(1, 0, N_DT)
                        emit_proj_v_mm(1)
                        emit_proj_v_tr(1)
                elif ci == 1:
                    if oi == 0:
                        emit_proj_qk(2, 0, 3)
                    elif oi == 1:
                        emit_proj_qk(2, 3, N_DT)
                        emit_proj_v_mm(2)
                        emit_proj_v_tr(2)
                elif ci + 1 < N_CH:
                    if oi == 0:
                        emit_proj_qk(ci + 1, 0, 3)
                    elif oi == 1:
                        emit_proj_qk(ci + 1, 3, N_DT)
                    elif oi == 2:
                        emit_proj_v_mm(ci + 1)
                    elif oi == 3:
                        emit_proj_v_tr(ci + 1)
            out_queue.append((cst, csz, pv))
        flush_pending()
        # final chunk: one copy + one DMA (staggered DMAs pay multiple
        # ~2us completion-sem latencies before the end-of-kernel barrier)
        flush_out()

    nc.compile()
    return nc


def _get(dt_mm_name=None):
    name = dt_mm_name or DT_MM_NAME
    if name not in _CACHE:
        _CACHE[name] = _build(name)
    return _CACHE[name]


def _ensure_axon_hooks():
    """The agent image's antenv lacks axon_hooks; bass_utils imports it when
    trace=True under axon. Provide it, wired to the real ctypes NTFF
    profiler from trn_agent_boot when available."""
    try:
        import antenv.axon_hooks  # noqa: F401

        return
    except ImportError:
        pass
    import types

    try:
        import antenv
    except ImportError:
        return
    mod = types.ModuleType("antenv.axon_hooks")
    mod._hook = None

    def set_axon_ntff_profile_hook(h):
        mod._hook = h

    def get_axon_ntff_profile_hook():
        return mod._hook

    mod.set_axon_ntff_profile_hook = set_axon_ntff_profile_hook
    mod.get_axon_ntff_profile_hook = get_axon_ntff_profile_hook
    sys.modules["antenv.axon_hooks"] = mod
    antenv.axon_hooks = mod
    try:
        from trn_agent_boot.trn_boot import _ntff_profile_via_ctypes

        so_path = "/opt/axon/libaxon_pjrt.so"
        if os.path.exists(so_path):
            mod._hook = _ntff_profile_via_ctypes(so_path)
    except Exception:
        pass


def kernel(x, mask, Wq, Wk, Wv):
    global LAST_RESULT
    _ensure_axon_hooks()
    from concourse.bass_utils import run_bass_kernel_spmd

    nc = _get()

    if DT_MM_NAME == "bfloat16":
        import ml_dtypes

        np_dt = ml_dtypes.bfloat16
    else:
        np_dt = np.float32

    x = np.asarray(x, dtype=np.float32)
    # [d, 192] = [Wq | Wk | Wv], partition-major [p, dt, e]
    w = np.concatenate(
        [np.asarray(t, dtype=np.float32) for t in (Wq, Wk, Wv)], axis=1
    )
    w_p = np.ascontiguousarray(
        w.reshape(N_DT, P, 3 * E).transpose(1, 0, 2)
    ).astype(np_dt)

    in_maps = []
    for b in range(BS):
        xT = x[b].T  # [d, s]
        m = {"W": w_p}
        for i, (st_, sz) in enumerate(CH):
            blk = xT[:, st_:st_ + sz].reshape(N_DT, P, sz)
            m["xc%d" % i] = np.ascontiguousarray(
                blk.transpose(1, 0, 2)
            ).astype(np_dt)
        in_maps.append(m)

    res = run_bass_kernel_spmd(nc, in_maps, core_ids=list(range(BS)))
    LAST_RESULT = res
    outs = []
    for b in range(BS):
        o = np.asarray(res.results[b]["out"]).astype(np.float32)
        outs.append((o[0:E] / o[E:E + 1]).T)
    return np.stack(outs, axis=0)
